# revision 1
# baseline (speedup 1.0000x reference)
"""CFConv (SchNet continuous-filter convolution) Trainium2 Bass kernel.

Problem: nn_CFConv_44332652429581 (gnn_message_passing, 8 cores).

Reference computation (per batch element b):
    W    = ssp(f_ij @ W1 + b1) @ W2 + b2          # filter net, (A,NBH,F)
    C    = 0.5*(cos(pi*r/5)+1)*(r<5)              # cosine cutoff, (A,NBH)
    Wc   = W * C * mask
    y    = x @ W_in2f                              # (A,F)
    agg  = sum_n  y[nbh[a,n]] * Wc[a,n]            # (A,F)
    out  = ssp(agg @ W_out + b_out)                # (A,O)
where ssp(v) = softplus(v) - ln 2.

Sharding: data-parallel over the batch axis, one batch element per core
(B=8 == n_cores). No collectives.

Per-core dataflow (pairs = A*NBH = 32768; "chunk" = one atom's 128
neighbors; "block" = 4096 pairs = 32 atoms). Blocks are pipelined:
load f_ij block / mm1 / ssp, gather that block's neighbor rows, then
filter-multiply-reduce it, all stages overlapping across blocks.

  - mm1 F-major: psum_h = W1^T @ f_ij^T            (bf16, W1 stationary)
  - ssp exactly in two ACT passes: u = Exp(h1+b1); h' = Ln(0.5u+0.5)
    (Ln(0.5e^v+0.5) == softplus(v) - ln2 exactly, so W = h'@W2 + b2)
  - y table (A,F) bf16 in DRAM; neighbor rows fetched with dma_gather,
    1024 rows per instruction (SWDGE ring holds 128 descs/engine)
    -> yg pairs-major, chunk == atom
  - mm2 per chunk: lhsT = h' slice (weights), rhs = W2 -> M2 (pairs,F) PSUM
  - V = yg * M2 on DVE (one pass per 512-pair superchunk)
  - neighbor reduce as one matmul per atom: aggT[:, a] = V_a^T @ Cm_a
    (the cutoff+mask vector Cm rides along as the reduce weights)
  - b2 correction (only when b2 != 0): agg += b2 (x)_f R, R^T = y^T @ T^T
    with T[a,j] = sum_{n: nbh[a,n]=j} Cm[a,n] host-precomputed from
    (neighbors, r_ij, mask) -- pure input preprocessing.
  - out-proj: psum_o = W_out^T @ aggT, final ssp via Exp/Ln,
    store outT (O, A); host transposes per core on unshard.

All DMA goes through gpsimd (SWDGE): the HWDGE rings (sync/scalar
engines) do not function on the axon PJRT runtime this kernel targets.

Performance notes (measured on HW via repeat-N slope, see test.py):
  - The neighbor gather is per-DESCRIPTOR bound (~10 ns per 256 B row per
    SWDGE queue, independent of row size).  The module is built with
    num_swdge_queues=4 and gathers spread queue_num=g%4: 3.3x faster
    than one queue.  This took the kernel from ~300 us to ~90 us/exec.
  - The exact 2-pass Exp/Ln ssp chain on ACT (~60 us busy) and the
    4-queue gather chain (~83 us) are balanced and fully overlapped.
  - The last block's Ln runs in 1024-col chunks so the mm2/V/reduce
    tail starts early; the out-projection runs in two halves (atoms
    0..127 after block 3) to shorten the serial tail.
"""
import math
import os

import numpy as np
import ml_dtypes

import concourse.bass as bass
import concourse.tile as tile
from concourse import bacc, mybir
from concourse.bass_utils import run_bass_kernel_spmd


def _patch_act_tables():
    """Prefer the combined Exp+Ln activation table so the ACT engine does
    not thrash 1.3us table reloads between the softplus Exp and Ln passes."""
    if getattr(bacc, "_cfconv_act_patch", False):
        return
    orig = bacc.get_activation_tables

    def patched(arch):
        # Table ids are positional: keep the dict order identical, but strip
        # Exp/Ln from the single-transcendental tables so the chooser must
        # pick the combined natural_log_exp table for both passes.
        t = dict(orig(arch))
        exp_t = mybir.ActivationFunctionType.Exp
        ln_t = mybir.ActivationFunctionType.Ln
        out = {}
        for k, funcs in t.items():
            if k != "natural_log_exp_and_others" and (
                    exp_t in funcs or ln_t in funcs):
                funcs = funcs - {exp_t, ln_t}
            out[k] = funcs
        return out

    bacc.get_activation_tables = patched
    bacc._cfconv_act_patch = True

F32 = mybir.dt.float32
BF16 = mybir.dt.bfloat16
I16 = mybir.dt.int16

B, A, NBH = 8, 256, 128
G, F, O = 25, 128, 128
PAIRS = A * NBH            # 32768
CUTOFF = 5.0
NBLK = 8                   # pipeline blocks of 4096 pairs
NSC_BLK = 8                # superchunks (512 pairs) per block
LN_SPLIT = True            # split last block's Ln for a shorter tail
CUT_POLY = True            # cutoff cosine as DVE polynomial (no ACT trig table)
RED_LAG = True             # lag reduce matmuls one superchunk behind mm2
MM1_AHEAD = False          # emit block i+1's mm1/ssp before block i's D stage
                           # (measured ~4us WORSE in-process at R=51 — the
                           # PE in-order queue was not the bottleneck; kept
                           # as a flag for reference)

# bf16 const-pack column layout
BC_XT = 0          # xT (128, 256)
BC_W2 = 256        # W2 (128, 128)
BC_WIN = 384       # W_in2f (128, 128)
BC_WOUT = 512      # W_out (128, 128)
BC_W1 = 640        # W1 padded to (128, 128); rows 0:25 valid
BC_TT = 768        # TT (2x (128, 256)) when use_b2

FC_RT = 0          # rT (128, 256) in the f32 pack

_prog_cache = {}
_runner_cache = {}
_last_results = None       # test.py introspection


def _build(use_b2: bool, use_mask: bool, n_repeat: int = 1,
           stages: frozenset | None = None):
    """Build + compile the per-core Bass program (SPMD; same for all cores).

    n_repeat > 1 repeats the ENTIRE body (constant loads included) that many
    times inside one NEFF.  Used only for timing: the per-execution device
    time is the slope of wall time vs n_repeat, which cancels the ~1.2 ms
    per-dispatch overhead of the axon PJRT tunnel.
    """
    from contextlib import ExitStack

    nbf = 768 + (512 if use_b2 else 0)
    fc_mask = 256
    fc_sc = 256 + (256 if use_mask else 0)   # scalar column block start
    nf32 = fc_sc + 5
    C_B1, C_BOUT, C_PIH, C_HALF, C_B2 = range(fc_sc, fc_sc + 5)

    _patch_act_tables()
    # 4 SWDGE queues: neighbor-row gathers are per-descriptor bound (~10 ns
    # per 256 B row on one queue, measured); four queues process descriptors
    # in parallel for a ~3.3x gather speedup.
    nc = bacc.Bacc("TRN2", num_swdge_queues=4)

    fijT_d = nc.dram_tensor("fijT", [G, PAIRS], BF16, kind="ExternalInput")
    idx_d = nc.dram_tensor("idx16", [128, PAIRS // 16], I16, kind="ExternalInput")
    bfp_d = nc.dram_tensor("bfpack", [128, nbf], BF16, kind="ExternalInput")
    fp_d = nc.dram_tensor("fpack", [128, nf32], F32, kind="ExternalInput")

    outT_d = nc.dram_tensor("outT", [O, A], F32, kind="ExternalOutput")
    ytab_d = nc.dram_tensor("ytab", [A, F], BF16)  # internal

    with tile.TileContext(nc) as tc:
        with ExitStack() as ctx:
            const = ctx.enter_context(tc.tile_pool(name="const", bufs=1))
            fpool = ctx.enter_context(tc.tile_pool(name="fij", bufs=4))
            upool = ctx.enter_context(tc.tile_pool(name="u", bufs=3))
            hpool = ctx.enter_context(tc.tile_pool(name="hh", bufs=3))
            ygpool = ctx.enter_context(tc.tile_pool(name="yg", bufs=3))
            vpool = ctx.enter_context(tc.tile_pool(name="v", bufs=6))
            mpool = ctx.enter_context(tc.tile_pool(name="misc", bufs=1))
            ps_h = ctx.enter_context(tc.tile_pool(name="psh", bufs=2, space="PSUM"))
            ps_m2 = ctx.enter_context(tc.tile_pool(name="psm2", bufs=2, space="PSUM"))
            ps_agg = ctx.enter_context(tc.tile_pool(name="psagg", bufs=1, space="PSUM"))
            ps_misc = ctx.enter_context(tc.tile_pool(name="psmisc", bufs=1, space="PSUM"))

            for _rep in range(n_repeat):
                _build_body(nc, tc, use_b2, use_mask,
                            fijT_d, idx_d, bfp_d, fp_d, outT_d, ytab_d,
                            const, fpool, upool, hpool, ygpool, vpool, mpool,
                            ps_h, ps_m2, ps_agg, ps_misc,
                            nbf, fc_mask, fc_sc,
                            C_B1, C_BOUT, C_PIH, C_HALF, C_B2, stages)

    nc.finalize()
    return nc


def _emit_out_half(nc, h, ps_aggT, wout_sb, bout_ap, half_ap,
                   mpool, ps_misc, outT_d):
    """Out-projection + exact final ssp for atoms h*128 .. h*128+127."""
    a0 = h * 128
    aggf = mpool.tile([128, 128], BF16, tag="aggf", name="aggf")
    nc.vector.tensor_copy(aggf[:], ps_aggT[:, a0:a0 + 128])
    po = ps_misc.tile([128, 128], F32, tag="pmisc", name="po")
    nc.tensor.matmul(po[:], wout_sb, aggf[:])
    u2 = mpool.tile([128, 128], F32, tag="u2", name="u2")
    nc.scalar.activation(u2[:], po[:], mybir.ActivationFunctionType.Exp,
                         bias=bout_ap, scale=1.0)
    oT = mpool.tile([128, 128], F32, tag="oT", name="oT")
    nc.scalar.activation(oT[:], u2[:], mybir.ActivationFunctionType.Ln,
                         bias=half_ap, scale=0.5)
    nc.gpsimd.dma_start(outT_d[:, a0:a0 + 128], oT[:])


def _build_body(nc, tc, use_b2, use_mask,
                fijT_d, idx_d, bfp_d, fp_d, outT_d, ytab_d,
                const, fpool, upool, hpool, ygpool, vpool, mpool,
                ps_h, ps_m2, ps_agg, ps_misc,
                nbf, fc_mask, fc_sc,
                C_B1, C_BOUT, C_PIH, C_HALF, C_B2, stages=None):
    nf32 = fc_sc + 5
    on = (lambda s: True) if stages is None else (lambda s: s in stages)
    if True:
        if True:
            # ---- packed constant loads (3 DMAs) ----
            # Order: bfp first (feeds the y-table matmul AND mm1 weights),
            # then f_ij block 0 (heads the ACT-critical mm1 chain), then the
            # fp scalars and the big idx pack (needed only once gathers
            # start, which is gated on the y table anyway).
            bfp = const.tile([128, nbf], BF16)
            nc.gpsimd.dma_start(bfp[:], bfp_d[:])
            if on('mm1'):
                fij_cur = fpool.tile([G, 4096], BF16)
                nc.gpsimd.dma_start(fij_cur[:], fijT_d[:, 0:4096])
            fp = const.tile([128, nf32], F32)
            nc.gpsimd.dma_start(fp[:], fp_d[:])
            idx_sb = const.tile([128, PAIRS // 16], I16)
            nc.gpsimd.dma_start(idx_sb[:], idx_d[:])

            xt_sb = bfp[:, BC_XT:BC_XT + 256]
            w2_sb = bfp[:, BC_W2:BC_W2 + 128]
            win_sb = bfp[:, BC_WIN:BC_WIN + 128]
            wout_sb = bfp[:, BC_WOUT:BC_WOUT + 128]
            w1_sb = bfp[0:G, BC_W1:BC_W1 + 128]
            rt_sb = fp[:, FC_RT:FC_RT + 256]
            b1_ap = fp[:, C_B1:C_B1 + 1]
            bout_ap = fp[:, C_BOUT:C_BOUT + 1]
            pih_ap = fp[:, C_PIH:C_PIH + 1]
            half_ap = fp[:, C_HALF:C_HALF + 1]

            # ---- stage A: y = x @ W_in2f  (atom-major, bf16 DRAM table) ----
            if not on('ytab'):
                y_sb = const.tile([128, 2, 128], BF16)
            if on('ytab'):
              psum_y = ps_misc.tile([128, 2, 128], F32, tag="pmisc")
              for ah in range(2):
                nc.tensor.matmul(
                    psum_y[:, ah, :],
                    xt_sb[:, ah * 128:(ah + 1) * 128],
                    win_sb,
                )
              y_sb = const.tile([128, 2, 128], BF16)
              nc.vector.tensor_copy(y_sb[:], psum_y[:])
              ytab_v = ytab_d[:].rearrange("(h p) f -> h p f", h=2)
              for ah in range(2):
                nc.gpsimd.dma_start(ytab_v[ah], y_sb[:, ah, :])

            # ---- stage A2: cutoff Cm (NBH-part, atom-free) ----
            if not on('cut'):
                cmb = const.tile([NBH, A], BF16)
            if on('cut') and not CUT_POLY:
              rcl = mpool.tile([NBH, A], F32)
              nc.vector.tensor_scalar(rcl[:], rt_sb, CUTOFF, None,
                                      op0=mybir.AluOpType.min)
              c1 = mpool.tile([NBH, A], F32)
              nc.scalar.activation(c1[:], rcl[:],
                                   mybir.ActivationFunctionType.Sin,
                                   bias=pih_ap, scale=float(-math.pi / CUTOFF))
              cm_f = mpool.tile([NBH, A], F32)
              nc.vector.tensor_scalar(cm_f[:], c1[:], 0.5, 0.5,
                                      op0=mybir.AluOpType.mult,
                                      op1=mybir.AluOpType.add)
              lt = mpool.tile([NBH, A], F32)
              nc.vector.tensor_scalar(lt[:], rt_sb, CUTOFF, None,
                                      op0=mybir.AluOpType.is_lt)
              nc.vector.tensor_tensor(cm_f[:], cm_f[:], lt[:],
                                      op=mybir.AluOpType.mult)
              if use_mask:
                  nc.vector.tensor_tensor(cm_f[:], cm_f[:],
                                          fp[:, fc_mask:fc_mask + 256],
                                          op=mybir.AluOpType.mult)
              cmb = const.tile([NBH, A], BF16)
              nc.vector.tensor_copy(cmb[:], cm_f[:])
            if on('cut') and CUT_POLY:
              # cos(pi*r/5) via a deg-9 Taylor of sin around pi/2, entirely
              # on DVE: keeps the ACT engine's combined Exp+Ln table loaded
              # for the whole program (no trig table round-trip, ~2.6us of
              # ACT-chain table reloads per repeat) and drops the early ACT
              # dependency on the fp scalar pack.  |error| < 4e-6, far below
              # the bf16 quantisation of cmb.
              #   u = min(r,5)*(pi/5) - pi/2             in [-pi/2, pi/2]
              #   0.5*(cos(pi*r/5)+1) = 0.5 + u*P(u^2),  P = -0.5*sin(u)/u
              rcl = mpool.tile([NBH, A], F32)
              nc.vector.tensor_scalar(rcl[:], rt_sb, CUTOFF, None,
                                      op0=mybir.AluOpType.min)
              u_c = mpool.tile([NBH, A], F32)
              nc.vector.tensor_scalar(u_c[:], rcl[:],
                                      float(math.pi / CUTOFF),
                                      float(-math.pi / 2),
                                      op0=mybir.AluOpType.mult,
                                      op1=mybir.AluOpType.add)
              u2_c = mpool.tile([NBH, A], F32)
              nc.vector.tensor_tensor(u2_c[:], u_c[:], u_c[:],
                                      op=mybir.AluOpType.mult)
              pp = mpool.tile([NBH, A], F32)
              nc.vector.tensor_scalar(pp[:], u2_c[:], -0.5 / 362880.0, None,
                                      op0=mybir.AluOpType.mult)
              for ck in (0.5 / 5040.0, -0.5 / 120.0, 0.5 / 6.0):
                  nc.vector.scalar_tensor_tensor(
                      out=pp[:], in0=pp[:], scalar=float(ck), in1=u2_c[:],
                      op0=mybir.AluOpType.add, op1=mybir.AluOpType.mult)
              nc.vector.tensor_scalar(pp[:], pp[:], -0.5, None,
                                      op0=mybir.AluOpType.add)
              qq = mpool.tile([NBH, A], F32)
              nc.vector.tensor_tensor(qq[:], pp[:], u_c[:],
                                      op=mybir.AluOpType.mult)
              lt = mpool.tile([NBH, A], F32)
              nc.vector.tensor_scalar(lt[:], rt_sb, CUTOFF, None,
                                      op0=mybir.AluOpType.is_lt)
              cmb = const.tile([NBH, A], BF16)
              if use_mask:
                  cm_f = mpool.tile([NBH, A], F32)
                  nc.vector.scalar_tensor_tensor(
                      out=cm_f[:], in0=qq[:], scalar=0.5, in1=lt[:],
                      op0=mybir.AluOpType.add, op1=mybir.AluOpType.mult)
                  nc.vector.tensor_tensor(cmb[:], cm_f[:],
                                          fp[:, fc_mask:fc_mask + 256],
                                          op=mybir.AluOpType.mult)
              else:
                  nc.vector.scalar_tensor_tensor(
                      out=cmb[:], in0=qq[:], scalar=0.5, in1=lt[:],
                      op0=mybir.AluOpType.add, op1=mybir.AluOpType.mult)

            # ---- pipelined blocks: mm1+ssp | gather | mm2+V+reduce ----
            # hh/yg are PER-BLOCK ring tiles (bufs=3), not monolithic
            # whole-pair buffers: ring reuse is tile-granular, so a
            # monolithic bufs=1 tile serialises each repeat's first write
            # against the previous repeat's LAST reader.  Per-block tiles
            # recycle within a repeat and decouple the repeat boundary
            # (and use 16 KB/partition instead of 128 KB).
            blk_hh = {}
            blk_yg = {}
            ps_aggT = ps_agg.tile([128, A], F32)

            fij_state = [fij_cur if on('mm1') else None]
            red_pend = [None]          # (v_t, sc) awaiting its reduce matmuls

            def emit_B(i):
                # B: mm1 + exact ssp on the prefetched f_ij block
                fij_t = fij_state[0]
                hh_sb = hpool.tile([128, 4096], BF16, name="hh_sb")
                blk_hh[i] = hh_sb
                u_t = upool.tile([128, 4096], BF16, name="u_t")
                for j in range(4):
                    ph = ps_h.tile([128, 1024], F32, name="ph")
                    for k in range(2):
                        o0 = j * 1024 + k * 512
                        nc.tensor.matmul(
                            ph[:, k * 512:(k + 1) * 512],
                            w1_sb,
                            fij_t[:, o0:o0 + 512],
                        )
                    nc.scalar.activation(u_t[:, j * 1024:(j + 1) * 1024],
                                         ph[:], mybir.ActivationFunctionType.Exp,
                                         bias=b1_ap, scale=1.0)
                # Last block: split the Ln into 1024-col chunks so the
                # D-stage (which reads hh per 512-col superchunk, subtile
                # deps) starts ~3 chunks earlier — shortens the serial tail.
                ln_chunks = 4 if (i == NBLK - 1 and LN_SPLIT) else 1
                for lc in range(ln_chunks):
                    w = 4096 // ln_chunks
                    nc.scalar.activation(
                        hh_sb[:, lc * w:(lc + 1) * w],
                        u_t[:, lc * w:(lc + 1) * w],
                        mybir.ActivationFunctionType.Ln,
                        bias=half_ap, scale=0.5)

                # prefetch the following block's f_ij ahead of the gathers
                # so the ACT-feeding mm1 chain never queues behind them
                if i + 1 < NBLK:
                    fij_nxt = fpool.tile([G, 4096], BF16, name="fij_cur")
                    nc.gpsimd.dma_start(
                        fij_nxt[:], fijT_d[:, (i + 1) * 4096:(i + 2) * 4096])
                    fij_state[0] = fij_nxt

            # Software-pipeline the mm1/ssp stage one block ahead of the
            # D stage: block i+1's mm1 matmuls are emitted BEFORE block i's
            # 64 D-stage matmuls, so on the in-order PE queue they never
            # wait behind them — the ACT chain (the critical path) stays fed.
            if on('mm1') and MM1_AHEAD:
                emit_B(0)
            for i in range(NBLK):
              if on('mm1') and (i + 1 < NBLK if MM1_AHEAD else True):
                emit_B(i + 1 if MM1_AHEAD else i)

              if on('gather'):
                # C: gather this block's 4096 neighbor rows, 1024 per
                # instruction (HW limit: 64 descriptors/engine per gather).
                yg_sb = ygpool.tile([128, 32, F], BF16, name="yg_sb")
                blk_yg[i] = yg_sb
                for g in range(4 * i, 4 * i + 4):
                    lg = g - 4 * i
                    nc.gpsimd.dma_gather(
                        out_ap=yg_sb[:, lg * 8:(lg + 1) * 8, :],
                        in_ap=ytab_d[:],
                        idxs_ap=idx_sb[:, g * 64:(g + 1) * 64],
                        num_idxs=1024,
                        num_idxs_reg=1024,
                        elem_size=F,
                        queue_num=g % 4,
                    )
              elif stages is not None and on('ygtouch'):
                yg_sb = ygpool.tile([128, 32, F], BF16, name="yg_sb")
                blk_yg[i] = yg_sb
                nc.gpsimd.memset(yg_sb[:, 0, :], 0.0)

              if on('mmv'):
                # D: filter-multiply-reduce for this block.  The reduce
                # matmuls for superchunk sc are emitted AFTER superchunk
                # sc+1's mm2 matmuls (lag one sc): PE is in-order, and a
                # reduce emitted right after its own mm2 would make PE sit
                # idle waiting for the DVE V-multiply before it may start
                # the next mm2 — serialising PE<->DVE every superchunk.
                for sc in range(NSC_BLK * i, NSC_BLK * (i + 1)):
                    pm2 = ps_m2.tile([128, 512], F32)
                    for k in range(4):
                        cl = 4 * (sc - NSC_BLK * i) + k
                        nc.tensor.matmul(
                            pm2[:, k * 128:(k + 1) * 128],
                            blk_hh[i][:, cl * 128:(cl + 1) * 128],
                            w2_sb,
                        )
                    if red_pend[0] is not None:
                        vp, scp = red_pend[0]
                        for k in range(4):
                            a = 4 * scp + k
                            nc.tensor.matmul(ps_aggT[:, a:a + 1], vp[:, k, :],
                                             cmb[:, a:a + 1])
                        red_pend[0] = None
                    scl = sc - NSC_BLK * i
                    v_t = vpool.tile([128, 4, 128], BF16)
                    nc.vector.tensor_tensor(
                        v_t[:], blk_yg[i][:, 4 * scl:4 * scl + 4, :],
                        pm2[:].rearrange("p (c f) -> p c f", f=128),
                        op=mybir.AluOpType.mult)
                    if RED_LAG:
                        red_pend[0] = (v_t, sc)
                    else:
                        for k in range(4):
                            a = 4 * sc + k
                            nc.tensor.matmul(ps_aggT[:, a:a + 1], v_t[:, k, :],
                                             cmb[:, a:a + 1])

              # out-projection for atoms 0..127 once their aggT columns are
              # complete (sc 0..31 reduced; with the lag-one reduce that is
              # after block 4's first mm2): halves the post-loop serial
              # tail.  Dependencies are tile-tracked, so this is only an
              # engine-queue placement choice.
              if on('out') and not use_b2 and i == 4:
                  _emit_out_half(nc, 0, ps_aggT, wout_sb, bout_ap, half_ap,
                                 mpool, ps_misc, outT_d)

            if on('mmv') and red_pend[0] is not None:
                vp, scp = red_pend[0]
                for k in range(4):
                    a = 4 * scp + k
                    nc.tensor.matmul(ps_aggT[:, a:a + 1], vp[:, k, :],
                                     cmb[:, a:a + 1])
                red_pend[0] = None

            # ---- stage E: b2 correction, out-projection, final ssp ----
            if not on('out'):
                return
            if not use_b2:
                _emit_out_half(nc, 1, ps_aggT, wout_sb, bout_ap, half_ap,
                               mpool, ps_misc, outT_d)
                return
            aggf = mpool.tile([128, A], BF16)
            if use_b2:
                prt = ps_misc.tile([128, A], F32, tag="pmisc")
                for jh in range(2):
                    nc.tensor.matmul(prt[:], y_sb[:, jh, :],
                                     bfp[:, BC_TT + jh * 256:BC_TT + (jh + 1) * 256],
                                     start=(jh == 0), stop=(jh == 1))
                rt2_sb = mpool.tile([128, A], F32)
                nc.vector.tensor_copy(rt2_sb[:], prt[:])
                nc.vector.scalar_tensor_tensor(
                    out=aggf[:], in0=rt2_sb[:], scalar=fp[:, C_B2:C_B2 + 1],
                    in1=ps_aggT[:],
                    op0=mybir.AluOpType.mult, op1=mybir.AluOpType.add)
            else:
                nc.vector.tensor_copy(aggf[:], ps_aggT[:])

            po = ps_misc.tile([128, A], F32, tag="pmisc")
            nc.tensor.matmul(po[:], wout_sb, aggf[:])
            u2 = mpool.tile([128, A], F32)
            nc.scalar.activation(u2[:], po[:], mybir.ActivationFunctionType.Exp,
                                 bias=bout_ap, scale=1.0)
            oT = mpool.tile([128, A], F32)
            nc.scalar.activation(oT[:], u2[:], mybir.ActivationFunctionType.Ln,
                                 bias=half_ap, scale=0.5)
            nc.gpsimd.dma_start(outT_d[:], oT[:])


def _make_runner(nc):
    """Jit the SPMD executable once; reuse across kernel() calls."""
    import jax
    from jax.sharding import Mesh, PartitionSpec
    from jax.experimental.shard_map import shard_map
    from concourse import bass2jax
    from concourse import mybir as mb

    bass2jax.install_neuronx_cc_hook()

    pid_name = nc.partition_id_tensor.name if nc.partition_id_tensor else None
    in_names, out_names, out_avals, zero_shapes = [], [], [], []
    for alloc in nc.m.functions[0].allocations:
        if not isinstance(alloc, mb.MemoryLocationSet):
            continue
        name = alloc.memorylocations[0].name
        if alloc.kind == "ExternalInput":
            if name != pid_name:
                in_names.append(name)
        elif alloc.kind == "ExternalOutput":
            shape = tuple(alloc.tensor_shape)
            dtype = mb.dt.np(alloc.dtype)
            out_names.append(name)
            out_avals.append(jax.core.ShapedArray(shape, dtype))
            zero_shapes.append((shape, dtype))
    n_params = len(in_names)
    all_in = in_names + out_names
    if pid_name is not None:
        all_in = all_in + [pid_name]
    donate = tuple(range(n_params, n_params + len(out_names)))

    def _body(*args):
        operands = list(args)
        if pid_name is not None:
            operands.append(bass2jax.partition_id_tensor())
        outs = bass2jax._bass_exec_p.bind(
            *operands,
            out_avals=tuple(out_avals),
            in_names=tuple(all_in),
            out_names=tuple(out_names),
            lowering_input_output_aliases=(),
            sim_require_finite=True,
            sim_require_nnan=True,
            nc=nc,
        )
        return tuple(outs)

    devices = jax.devices()[:B]
    mesh = Mesh(np.asarray(devices), ("core",))
    nin = n_params + len(out_names)
    sharded = jax.jit(
        shard_map(_body, mesh=mesh,
                  in_specs=(PartitionSpec("core"),) * nin,
                  out_specs=(PartitionSpec("core"),) * len(out_names),
                  check_rep=False),
        donate_argnums=donate, keep_unused=True)

    def run(in_maps):
        concat_in = [
            np.concatenate([np.asarray(in_maps[c][n]) for c in range(B)], axis=0)
            for n in in_names
        ]
        zeros = [np.zeros((B * s[0], *s[1:]), d) for s, d in zero_shapes]
        out_arrs = sharded(*concat_in, *zeros)
        return [
            {n: np.asarray(out_arrs[i]).reshape(B, *out_avals[i].shape)[c]
             for i, n in enumerate(out_names)}
            for c in range(B)
        ]

    run.sharded = sharded
    run.in_names = in_names
    run.zero_shapes = zero_shapes
    return run


def _prep_shared(W1, W2, W_in2f, W_out, b1, b2, b_out, use_b2, use_mask):
    nbf = 768 + (512 if use_b2 else 0)
    bfp = np.zeros((128, nbf), ml_dtypes.bfloat16)
    bfp[:, BC_W2:BC_W2 + 128] = W2.astype(ml_dtypes.bfloat16)
    bfp[:, BC_WIN:BC_WIN + 128] = W_in2f.astype(ml_dtypes.bfloat16)
    bfp[:, BC_WOUT:BC_WOUT + 128] = W_out.astype(ml_dtypes.bfloat16)
    bfp[0:G, BC_W1:BC_W1 + 128] = W1.astype(ml_dtypes.bfloat16)

    fc_sc = 256 + (256 if use_mask else 0)
    nf32 = fc_sc + 5
    fp = np.zeros((128, nf32), np.float32)
    fp[:, fc_sc + 0] = b1
    fp[:, fc_sc + 1] = b_out
    fp[:, fc_sc + 2] = np.pi / 2
    fp[:, fc_sc + 3] = 0.5
    if use_b2:
        fp[:, fc_sc + 4] = b2
    return bfp, fp, fc_sc


def _prep_core(b, x, r_ij, nbh, mask, f_ij, bfp, fp, use_b2, use_mask):
    """Host-side per-core input marshalling (layout only + index preproc)."""
    m = {}
    m["fijT"] = np.ascontiguousarray(
        f_ij[b].reshape(PAIRS, G).T).astype(ml_dtypes.bfloat16)
    flat = nbh[b].reshape(PAIRS).astype(np.int16)
    idx16 = np.ascontiguousarray(flat.reshape(PAIRS // 16, 16).T)  # (16, 2048)
    m["idx16"] = np.tile(idx16, (8, 1))

    bfp_c = bfp.copy()
    bfp_c[:, BC_XT:BC_XT + 256] = x[b].T.astype(ml_dtypes.bfloat16)
    if use_b2:
        cm = (0.5 * (np.cos(r_ij[b] * (np.pi / CUTOFF)) + 1.0)
              * (r_ij[b] < CUTOFF) * mask[b]).astype(np.float32)  # (A, NBH)
        T = np.zeros((A, A), np.float32)
        np.add.at(T, (np.repeat(np.arange(A), NBH), nbh[b].reshape(-1)),
                  cm.reshape(-1))
        bfp_c[:, BC_TT:BC_TT + 512] = np.concatenate(
            [T.T[0:128], T.T[128:256]], axis=1).astype(ml_dtypes.bfloat16)
    m["bfpack"] = bfp_c

    fp_c = fp.copy()
    fp_c[:, FC_RT:FC_RT + 256] = r_ij[b].T.astype(np.float32)
    if use_mask:
        fp_c[:, 256:512] = mask[b].T.astype(np.float32)
    m["fpack"] = fp_c
    return m


def kernel(**inputs) -> np.ndarray:
    global _last_results
    x = np.asarray(inputs["x"], np.float32)
    r_ij = np.asarray(inputs["r_ij"], np.float32)
    nbh = np.asarray(inputs["neighbors"])
    mask = np.asarray(inputs["pairwise_mask"], np.float32)
    f_ij = np.asarray(inputs["f_ij"], np.float32)
    W1 = np.asarray(inputs["W1"], np.float32)
    b1 = np.asarray(inputs["b1"], np.float32)
    W2 = np.asarray(inputs["W2"], np.float32)
    b2 = np.asarray(inputs["b2"], np.float32)
    W_in2f = np.asarray(inputs["W_in2f"], np.float32)
    W_out = np.asarray(inputs["W_out"], np.float32)
    b_out = np.asarray(inputs["b_out"], np.float32)

    use_b2 = bool(np.any(b2 != 0.0))
    use_mask = bool(np.any(mask != 1.0))

    key = (use_b2, use_mask)
    if key not in _prog_cache:
        _prog_cache[key] = _build(use_b2, use_mask)
    nc = _prog_cache[key]
    if key not in _runner_cache:
        _runner_cache[key] = _make_runner(nc)
    runner = _runner_cache[key]

    bfp, fp, _ = _prep_shared(W1, W2, W_in2f, W_out, b1, b2, b_out,
                              use_b2, use_mask)
    in_maps = [
        _prep_core(b, x, r_ij, nbh, mask, f_ij, bfp, fp, use_b2, use_mask)
        for b in range(B)
    ]

    if os.environ.get("CFCONV_TRACE"):
        res = run_bass_kernel_spmd(nc, in_maps, list(range(B)), trace=True)
        _last_results = res
        results = res.results
    else:
        results = runner(in_maps)
    out = np.stack([np.asarray(results[b]["outT"]).T for b in range(B)])
    return out.astype(np.float32)

